# revision 1
# baseline (speedup 1.0000x reference)
"""LocalMeanInpainter Trainium2 kernel.

out = x*mask + (box15(x)/box15(ones))*(1-mask)  over (32,3,512,512) f32.

Strategy: data-parallel over batch (4 images x 3 channels = 12 planes of
512x512 per core, 8 cores). Per plane, the 15x15 box mean is separable:
mean = diag(1/ch) @ B @ X @ B @ diag(1/cw) with B the 0/1 banded matrix
(|i-j|<=7) and ch/cw the 1-D in-bounds counts (cnt = outer(ch, cw) exactly).
Both passes run on the PE tensor engine with the normalization folded into
the B weights:
  pass1: S1T[w, h_out] = sum_h X[h, w] * BH[h, h_out]   (X chunk stationary)
  pass2: S2[h_out, w_out] = sum_w S1T[w, h_out] * BW[w, w_out]
Blend: mask is exactly {0,1} (shipped as uint8), so out = select(mask, x,
mean): DVE tensor_copy from PSUM + copy_predicated, then DMA out.
"""

import numpy as np
import ml_dtypes

H = 512
W = 512
WINDOW = 15
PAD = 7
N_CORES = 8
IMGS_PER_CORE = 4
CHANNELS = 3
PLANES = IMGS_PER_CORE * CHANNELS  # 12
NCHUNK = H // 128  # 4

# matmul operand dtype: "f32r" = fp32 bits in the PE's full-rate replicated
# mode (no x cast needed), "bf16" = cast x/S1 to bf16 first.
MM_DTYPE = "f32r"
BANDED = False
GROUP = 1

_CACHE = {}


def _band_matrix(n, normalize_cols):
    idx = np.arange(n)
    band = (np.abs(idx[:, None] - idx[None, :]) <= PAD).astype(np.float64)
    if normalize_cols:
        cnt = np.minimum(idx + PAD, n - 1) - np.maximum(idx - PAD, 0) + 1
        band = band / cnt[None, :]
    return band


def _build_program(planes=PLANES, reps=1, mm_dtype=None, banded=None, ablate=()):
    import concourse.tile as tile
    from concourse import bacc, mybir
    from contextlib import nullcontext

    if mm_dtype is None:
        mm_dtype = MM_DTYPE
    if banded is None:
        banded = BANDED
    f32 = mybir.dt.float32
    use_f32r = mm_dtype == "f32r"
    w_dt = mybir.dt.float32r if use_f32r else mybir.dt.bfloat16
    s1_dt = mybir.dt.float32r if use_f32r else mybir.dt.bfloat16

    nc = bacc.Bacc("TRN2", target_bir_lowering=False, debug=False, num_devices=N_CORES)
    x_dt = mybir.dt.float32r if use_f32r else f32
    x_d = nc.declare_dram_parameter("x", [planes, H, W], x_dt, isOutput=False)
    m_d = nc.declare_dram_parameter(
        "mask", [planes, H, W], mybir.dt.uint8, isOutput=False
    )
    bh_d = nc.declare_dram_parameter("bh", [H, H], w_dt, isOutput=False)
    bw_d = nc.declare_dram_parameter("bw", [W, W], w_dt, isOutput=False)
    out_d = nc.declare_dram_parameter("out", [planes, H, W], f32, isOutput=True)

    def as_mm(ap):
        return ap.bitcast(mybir.dt.float32r) if use_f32r else ap

    with tile.TileContext(nc) as tc:
        with (
            tc.tile_pool(name="consts", bufs=1) as cpool,
            tc.tile_pool(name="xt", bufs=6) as xpool,
            tc.tile_pool(name="mt", bufs=6) as mpool,
            tc.tile_pool(name="xb", bufs=3) as xbpool,
            tc.tile_pool(name="s1b", bufs=4) as s1pool,
            tc.tile_pool(name="ot", bufs=6) as opool,
            tc.tile_pool(name="ps1", bufs=3, space="PSUM") as ps1pool,
            tc.tile_pool(name="ps2", bufs=5, space="PSUM") as ps2pool,
        ):
            # B constants: stored [128, (chunk, 512)] — partition = row within
            # chunk, free slice c selects row-chunk c.
            bh_t = cpool.tile([128, NCHUNK * H], w_dt, tag="bh")
            nc.sync.dma_start(
                out=bh_t[:].rearrange("h (c n) -> h c n", c=NCHUNK),
                in_=bh_d[:].rearrange("(c h) n -> h c n", c=NCHUNK),
            )
            bw_t = cpool.tile([128, NCHUNK * W], w_dt, tag="bw")
            nc.sync.dma_start(
                out=bw_t[:].rearrange("h (c n) -> h c n", c=NCHUNK),
                in_=bw_d[:].rearrange("(c h) n -> h c n", c=NCHUNK),
            )

            def mms(ps, lhsT_of, rhs_tile, rhs_base):
                if not banded:
                    for kc in range(NCHUNK):
                        nc.tensor.matmul(
                            ps[:],
                            lhsT=lhsT_of(kc),
                            rhs=rhs_tile[:, rhs_base(kc) : rhs_base(kc) + 512],
                            start=(kc == 0),
                            stop=(kc == NCHUNK - 1),
                        )
                    return
                # Banded: chunk kc only touches output cols [128k-7, 128k+135).
                for kc in range(NCHUNK):
                    lo, hi = 128 * kc, 128 * (kc + 1)
                    segs = []
                    if kc > 0:
                        segs.append((lo - PAD, lo + PAD, False, True))
                    e0 = lo if kc == 0 else lo + PAD
                    e1 = hi if kc == NCHUNK - 1 else hi - PAD
                    segs.append((e0, e1, True, True))
                    if kc < NCHUNK - 1:
                        segs.append((hi - PAD, hi + PAD, True, False))
                    lhsT = lhsT_of(kc)
                    for c0, c1, st, sp in segs:
                        nc.tensor.matmul(
                            ps[:, c0:c1],
                            lhsT=lhsT,
                            rhs=rhs_tile[:, rhs_base(kc) + c0 : rhs_base(kc) + c1],
                            start=st,
                            stop=sp,
                        )

            loop_ctx = (
                tc.For_i(
                    0,
                    reps,
                    1,
                    hint_engines=tuple(
                        getattr(mybir.EngineType, e)
                        for e in ("PE", "Activation", "DVE", "SP", "Pool")
                    ),
                )
                if reps > 1
                else nullcontext()
            )
            G = GROUP if planes % GROUP == 0 else 1
            with loop_ctx:
              for pg in range(planes // G):
                # load G planes per DMA (bigger transfers = higher HBM eff.)
                xt = xpool.tile([128, G * NCHUNK * W], x_dt, tag="xt")
                if "no_in_dma" not in ablate:
                    nc.sync.dma_start(
                        out=xt[:].rearrange("h (g c w) -> h (g c) w", g=G, c=NCHUNK),
                        in_=x_d[pg * G : (pg + 1) * G]
                        .rearrange("g (c h) w -> h g c w", c=NCHUNK)
                        .rearrange("h g c w -> h (g c) w"),
                    )
                mt = mpool.tile([128, G * NCHUNK * W], mybir.dt.uint8, tag="mt")
                if "no_in_dma" not in ablate:
                    nc.sync.dma_start(
                        out=mt[:].rearrange("h (g c w) -> h (g c) w", g=G, c=NCHUNK),
                        in_=m_d[pg * G : (pg + 1) * G]
                        .rearrange("g (c h) w -> h g c w", c=NCHUNK)
                        .rearrange("h g c w -> h (g c) w"),
                    )
                for g in range(G):
                    p = pg * G + g
                    xtg = xt[:, g * NCHUNK * W : (g + 1) * NCHUNK * W]
                    mtg = mt[:, g * NCHUNK * W : (g + 1) * NCHUNK * W]
                    if use_f32r:
                        xmm = xtg
                    else:
                        xmm = xbpool.tile([128, NCHUNK * W], s1_dt, tag="xb")
                        for c in range(NCHUNK):
                            nc.scalar.copy(
                                xmm[:, c * W : (c + 1) * W],
                                xtg[:, c * W : (c + 1) * W],
                            )

                    # pass 1: S1T[wc] [128 w, 512 h_out] over h chunks
                    s1b = s1pool.tile([128, NCHUNK * H], s1_dt, tag="s1b")
                    for wc in ([] if "no_pe" in ablate else range(NCHUNK)):
                        ps1 = ps1pool.tile([128, H], f32, tag="ps1")
                        mms(
                            ps1,
                            lambda kc: as_mm(
                                xmm[:, kc * W + wc * 128 : kc * W + wc * 128 + 128]
                            ),
                            bh_t,
                            lambda kc: kc * H,
                        )
                        nc.scalar.copy(s1b[:, wc * H : (wc + 1) * H], ps1[:])

                    # pass 2: S2[mc] [128 h_out, 512 w_out] over w chunks;
                    # blend into a full-plane out tile, one 1MB DMA per plane
                    ot = opool.tile([128, NCHUNK * W], f32, tag="ot")
                    for mc in range(NCHUNK):
                        ps2 = ps2pool.tile([128, W], f32, tag="ps2")
                        if "no_pe" not in ablate:
                            mms(
                                ps2,
                                lambda kc: as_mm(
                                    s1b[:, kc * H + mc * 128 : kc * H + mc * 128 + 128]
                                ),
                                bw_t,
                                lambda kc: kc * W,
                            )
                        otm = ot[:, mc * W : (mc + 1) * W]
                        if "no_pe" in ablate:
                            nc.scalar.copy(otm, xtg[:, mc * W : (mc + 1) * W].bitcast(f32))
                        else:
                            nc.scalar.copy(otm, ps2[:])
                        nc.vector.copy_predicated(
                            otm,
                            mtg[:, mc * W : (mc + 1) * W],
                            xtg[:, mc * W : (mc + 1) * W].bitcast(f32),
                        )
                        op_idx = 0 if "out_same" in ablate else p
                        nc.sync.dma_start(
                            out=out_d[op_idx, mc * 128 : (mc + 1) * 128, :],
                            in_=otm,
                        )
    nc.finalize()
    return nc


def _host_weights(mm_dtype=None):
    if mm_dtype is None:
        mm_dtype = MM_DTYPE
    wt = np.float32 if mm_dtype == "f32r" else ml_dtypes.bfloat16
    return (
        _band_matrix(H, True).astype(wt),
        _band_matrix(W, True).astype(wt),
    )


def _get_program():
    if "nc" not in _CACHE:
        _CACHE["nc"] = _build_program()
        _CACHE["bh"], _CACHE["bw"] = _host_weights()
    return _CACHE["nc"], _CACHE["bh"], _CACHE["bw"]


def kernel(x: np.ndarray, mask: np.ndarray) -> np.ndarray:
    from concourse.bass_utils import run_bass_kernel_spmd

    nc, bh, bw = _get_program()

    x = np.ascontiguousarray(x, dtype=np.float32)
    mask = np.ascontiguousarray(mask).astype(np.uint8)
    xs = x.reshape(N_CORES, PLANES, H, W)
    ms = mask.reshape(N_CORES, PLANES, H, W)

    in_maps = [
        {"x": xs[i], "mask": ms[i], "bh": bh, "bw": bw} for i in range(N_CORES)
    ]
    res = run_bass_kernel_spmd(nc, in_maps, core_ids=list(range(N_CORES)))
    out = np.stack([res.results[i]["out"] for i in range(N_CORES)])
    return out.reshape(x.shape[0] // IMGS_PER_CORE, IMGS_PER_CORE, CHANNELS, H, W).reshape(
        -1, CHANNELS, H, W
    )



# revision 2
# speedup vs baseline: 1.6593x; 1.6593x over previous
"""LocalMeanInpainter Trainium2 kernel.

out = x*mask + (box15(x)/box15(ones))*(1-mask)  over (32,3,512,512) f32.

Strategy: data-parallel over batch (4 images x 3 channels = 12 planes of
512x512 per core, 8 cores). Per plane, the 15x15 box mean is separable:
mean = BHn^T @ X @ BWn with BHn/BWn the 0/1 banded matrices (|i-j|<=7)
column-normalized by the 1-D in-bounds counts (cnt = outer(ch, cw) exactly).
Both passes run on the PE tensor engine:
  pass1: S1T[w, h_out] = sum_h X[h, w] * BH[h, h_out]   (X chunk stationary)
  pass2: S2[h_out, w_out] = sum_w S1T[w, h_out] * BW[w, w_out]

Perf notes (vs the 104us f32r baseline):
 - x / out / B-weights travel as fp16 (tolerance is 2e-2; fp16 adds ~1e-4):
   halves the dominant HBM traffic (28.3MB -> 15.7MB per core per rep).
 - banded matmuls: only the |i-j|<=7 band of B contributes, so each
   contraction chunk only touches ~139 of 512 output columns (fp16 runs
   1 cycle/row at any width; f32r needed >=256-wide to hit full rate,
   which is why the old kernel kept dense matmuls).
 - blend is done IN PLACE into the x tile: copy_predicated overwrites x
   with the PSUM mean where the (host-inverted) mask says "missing", then
   the x tile is DMA'd out directly. No staging copy of the mean.
"""

import numpy as np
import ml_dtypes

H = 512
W = 512
WINDOW = 15
PAD = 7
N_CORES = 8
IMGS_PER_CORE = 4
CHANNELS = 3
PLANES = IMGS_PER_CORE * CHANNELS  # 12
NCHUNK = H // 128  # 4

MM_DTYPE = "f16"  # "f16" | "bf16"
BANDED = True
BLEND = "inplace"  # "inplace" | "staged"
GROUP = 1

_CACHE = {}


def _band_matrix(n, normalize_cols):
    idx = np.arange(n)
    band = (np.abs(idx[:, None] - idx[None, :]) <= PAD).astype(np.float64)
    if normalize_cols:
        cnt = np.minimum(idx + PAD, n - 1) - np.maximum(idx - PAD, 0) + 1
        band = band / cnt[None, :]
    return band


def _build_program(planes=PLANES, reps=1, mm_dtype=None, banded=None, blend=None,
                   ablate=()):
    import concourse.tile as tile
    from concourse import bacc, mybir
    from contextlib import nullcontext

    if mm_dtype is None:
        mm_dtype = MM_DTYPE
    if banded is None:
        banded = BANDED
    if blend is None:
        blend = BLEND
    f32 = mybir.dt.float32
    dt16 = mybir.dt.float16 if mm_dtype == "f16" else mybir.dt.bfloat16

    nc = bacc.Bacc("TRN2", target_bir_lowering=False, debug=False, num_devices=N_CORES)
    x_d = nc.declare_dram_parameter("x", [planes, H, W], dt16, isOutput=False)
    # NOTE: for blend=="inplace" the host ships the INVERTED mask
    # (1 = missing pixel = overwrite with mean).
    m_d = nc.declare_dram_parameter(
        "mask", [planes, H, W], mybir.dt.uint8, isOutput=False
    )
    bh_d = nc.declare_dram_parameter("bh", [H, H], dt16, isOutput=False)
    bw_d = nc.declare_dram_parameter("bw", [W, W], dt16, isOutput=False)
    out_d = nc.declare_dram_parameter("out", [planes, H, W], dt16, isOutput=True)

    with tile.TileContext(nc) as tc:
        with (
            tc.tile_pool(name="consts", bufs=1) as cpool,
            tc.tile_pool(name="xt", bufs=6) as xpool,
            tc.tile_pool(name="mt", bufs=6) as mpool,
            tc.tile_pool(name="s1b", bufs=4) as s1pool,
            tc.tile_pool(name="ot", bufs=6) as opool,
            tc.tile_pool(name="ps1", bufs=3, space="PSUM") as ps1pool,
            tc.tile_pool(name="ps2", bufs=5, space="PSUM") as ps2pool,
        ):
            # B constants: stored [128, (chunk, 512)] — partition = row within
            # chunk, free slice c selects row-chunk c.
            bh_t = cpool.tile([128, NCHUNK * H], dt16, tag="bh")
            nc.sync.dma_start(
                out=bh_t[:].rearrange("h (c n) -> h c n", c=NCHUNK),
                in_=bh_d[:].rearrange("(c h) n -> h c n", c=NCHUNK),
            )
            bw_t = cpool.tile([128, NCHUNK * W], dt16, tag="bw")
            nc.sync.dma_start(
                out=bw_t[:].rearrange("h (c n) -> h c n", c=NCHUNK),
                in_=bw_d[:].rearrange("(c h) n -> h c n", c=NCHUNK),
            )

            def mms(ps, lhsT_of, rhs_tile, rhs_base):
                if not banded:
                    for kc in range(NCHUNK):
                        nc.tensor.matmul(
                            ps[:],
                            lhsT=lhsT_of(kc),
                            rhs=rhs_tile[:, rhs_base(kc) : rhs_base(kc) + 512],
                            start=(kc == 0),
                            stop=(kc == NCHUNK - 1),
                        )
                    return
                # Banded: chunk kc only touches output cols [128k-7, 128k+135).
                for kc in range(NCHUNK):
                    lo, hi = 128 * kc, 128 * (kc + 1)
                    segs = []
                    if kc > 0:
                        segs.append((lo - PAD, lo + PAD, False, True))
                    e0 = lo if kc == 0 else lo + PAD
                    e1 = hi if kc == NCHUNK - 1 else hi - PAD
                    segs.append((e0, e1, True, True))
                    if kc < NCHUNK - 1:
                        segs.append((hi - PAD, hi + PAD, True, False))
                    lhsT = lhsT_of(kc)
                    for c0, c1, st, sp in segs:
                        nc.tensor.matmul(
                            ps[:, c0:c1],
                            lhsT=lhsT,
                            rhs=rhs_tile[:, rhs_base(kc) + c0 : rhs_base(kc) + c1],
                            start=st,
                            stop=sp,
                        )

            loop_ctx = (
                tc.For_i(
                    0,
                    reps,
                    1,
                    hint_engines=tuple(
                        getattr(mybir.EngineType, e)
                        for e in ("PE", "Activation", "DVE", "SP", "Pool")
                    ),
                )
                if reps > 1
                else nullcontext()
            )
            G = GROUP if planes % GROUP == 0 else 1
            with loop_ctx:
              for pg in range(planes // G):
                xt = xpool.tile([128, G * NCHUNK * W], dt16, tag="xt")
                if "no_in_dma" not in ablate:
                    nc.sync.dma_start(
                        out=xt[:].rearrange("h (g c w) -> h (g c) w", g=G, c=NCHUNK),
                        in_=x_d[pg * G : (pg + 1) * G]
                        .rearrange("g (c h) w -> h g c w", c=NCHUNK)
                        .rearrange("h g c w -> h (g c) w"),
                    )
                mt = mpool.tile([128, G * NCHUNK * W], mybir.dt.uint8, tag="mt")
                if "no_in_dma" not in ablate:
                    nc.sync.dma_start(
                        out=mt[:].rearrange("h (g c w) -> h (g c) w", g=G, c=NCHUNK),
                        in_=m_d[pg * G : (pg + 1) * G]
                        .rearrange("g (c h) w -> h g c w", c=NCHUNK)
                        .rearrange("h g c w -> h (g c) w"),
                    )
                for g in range(G):
                    p = pg * G + g
                    xtg = xt[:, g * NCHUNK * W : (g + 1) * NCHUNK * W]
                    mtg = mt[:, g * NCHUNK * W : (g + 1) * NCHUNK * W]

                    # pass 1: S1T[wc] [128 w, 512 h_out] over h chunks
                    s1b = s1pool.tile([128, NCHUNK * H], dt16, tag="s1b")
                    for wc in ([] if "no_pe" in ablate else range(NCHUNK)):
                        ps1 = ps1pool.tile([128, H], f32, tag="ps1")
                        mms(
                            ps1,
                            lambda kc: xtg[:, kc * W + wc * 128 : kc * W + wc * 128 + 128],
                            bh_t,
                            lambda kc: kc * H,
                        )
                        nc.scalar.copy(s1b[:, wc * H : (wc + 1) * H], ps1[:])

                    # pass 2: S2[mc] [128 h_out, 512 w_out] over w chunks,
                    # then blend + store.
                    if blend == "staged":
                        ot = opool.tile([128, NCHUNK * W], dt16, tag="ot")
                    for mc in range(NCHUNK):
                        ps2 = ps2pool.tile([128, W], f32, tag="ps2")
                        if "no_pe" not in ablate:
                            mms(
                                ps2,
                                lambda kc: s1b[:, kc * H + mc * 128 : kc * H + mc * 128 + 128],
                                bw_t,
                                lambda kc: kc * W,
                            )
                        if blend == "inplace":
                            # mask here is INVERTED: overwrite x with the mean
                            # where the pixel is missing; result lands in xtg.
                            if "no_pe" not in ablate:
                                nc.vector.copy_predicated(
                                    xtg[:, mc * W : (mc + 1) * W],
                                    mtg[:, mc * W : (mc + 1) * W],
                                    ps2[:],
                                )
                        else:
                            otm = ot[:, mc * W : (mc + 1) * W]
                            # mean -> ot (split between Act and Pool engines),
                            # then overwrite kept pixels with x on DVE.
                            if "no_pe" in ablate:
                                nc.scalar.copy(otm, xtg[:, mc * W : (mc + 1) * W])
                            elif mc == 0:
                                nc.scalar.copy(otm, ps2[:])
                            else:
                                nc.gpsimd.tensor_copy(otm, ps2[:])
                            nc.vector.copy_predicated(
                                otm,
                                mtg[:, mc * W : (mc + 1) * W],
                                xtg[:, mc * W : (mc + 1) * W],
                            )
                    src = xtg if blend == "inplace" else ot[:]
                    op_idx = 0 if "out_same" in ablate else p
                    nc.sync.dma_start(
                        out=out_d[op_idx].rearrange("(c h) w -> h c w", c=NCHUNK),
                        in_=src.rearrange("h (c w) -> h c w", c=NCHUNK),
                    )
    nc.finalize()
    return nc


def _host_weights(mm_dtype=None):
    if mm_dtype is None:
        mm_dtype = MM_DTYPE
    wt = np.float16 if mm_dtype == "f16" else ml_dtypes.bfloat16
    return (
        _band_matrix(H, True).astype(wt),
        _band_matrix(W, True).astype(wt),
    )


def _get_program():
    if "nc" not in _CACHE:
        _CACHE["nc"] = _build_program()
        _CACHE["bh"], _CACHE["bw"] = _host_weights()
    return _CACHE["nc"], _CACHE["bh"], _CACHE["bw"]


def _make_in_maps(x, mask, mm_dtype=None, blend=None):
    """Shard + convert FULL inputs into per-core device input dicts."""
    if mm_dtype is None:
        mm_dtype = MM_DTYPE
    if blend is None:
        blend = BLEND
    wt = np.float16 if mm_dtype == "f16" else ml_dtypes.bfloat16
    x16 = np.ascontiguousarray(x).astype(wt)
    if blend == "inplace":
        m8 = (np.ascontiguousarray(mask) == 0).astype(np.uint8)
    else:
        m8 = np.ascontiguousarray(mask).astype(np.uint8)
    bh, bw = _host_weights(mm_dtype)
    xs = x16.reshape(N_CORES, PLANES, H, W)
    ms = m8.reshape(N_CORES, PLANES, H, W)
    return [
        {"x": xs[i], "mask": ms[i], "bh": bh, "bw": bw} for i in range(N_CORES)
    ]


def kernel(x: np.ndarray, mask: np.ndarray) -> np.ndarray:
    from concourse.bass_utils import run_bass_kernel_spmd

    nc, _, _ = _get_program()
    in_maps = _make_in_maps(x, mask)
    res = run_bass_kernel_spmd(nc, in_maps, core_ids=list(range(N_CORES)))
    out = np.stack([res.results[i]["out"].astype(np.float32) for i in range(N_CORES)])
    return out.reshape(x.shape[0] // IMGS_PER_CORE, IMGS_PER_CORE, CHANNELS, H, W).reshape(
        -1, CHANNELS, H, W
    )


# revision 3
# speedup vs baseline: 1.8741x; 1.1295x over previous
"""LocalMeanInpainter Trainium2 kernel.

out = x*mask + (box15(x)/box15(ones))*(1-mask)  over (32,3,512,512) f32.

Strategy: data-parallel over batch (4 images x 3 channels = 12 planes of
512x512 per core, 8 cores). Per plane, the 15x15 box mean is separable:
mean = BHn^T @ X @ BWn with BHn/BWn the 0/1 banded matrices (|i-j|<=7)
column-normalized by the 1-D in-bounds counts (cnt = outer(ch, cw) exactly).
Both passes run on the PE tensor engine:
  pass1: S1T[w, h_out] = sum_h X[h, w] * BH[h, h_out]   (X chunk stationary)
  pass2: S2[h_out, w_out] = sum_w S1T[w, h_out] * BW[w, w_out]

Perf notes (HW-measured path: 104us f32r-dense baseline -> 63us fp16-banded
-> this version). The kernel is DMA-bound, so everything targets HBM bytes:
 - x / out / B-weights travel as fp16 (tolerance is 2e-2; fp16 adds ~2e-4).
 - mask travels as the LSB of x's fp16 mantissa (host-encoded): missing
   pixels have bit0 set, kept pixels bit0 cleared (<=1ulp noise). A single
   DVE tensor_scalar(bitwise_and) per plane recovers the predicate, so the
   3.1MB/core uint8 mask DMA disappears entirely.
 - host ships planes in h-major layout [plane, h(128), chunk, w] so every
   plane is ONE [128 x 4KB-contiguous] DMA in and out.
 - banded matmuls: only the |i-j|<=7 band of B contributes, so each
   contraction chunk only touches ~139 of 512 output columns (fp16 runs
   1 cycle/row at any width; f32r needed >=256-wide to hit full rate).
 - blend is done IN PLACE into the x tile: copy_predicated overwrites x
   with the PSUM mean where the mask bit says "missing", then the x tile
   is DMA'd out directly. No staging copy of the mean.
"""

import numpy as np
import ml_dtypes

H = 512
W = 512
WINDOW = 15
PAD = 7
N_CORES = 8
IMGS_PER_CORE = 4
CHANNELS = 3
PLANES = IMGS_PER_CORE * CHANNELS  # 12
NCHUNK = H // 128  # 4
FREE = NCHUNK * W  # 2048 free elems per partition per plane

MM_DTYPE = "f16"  # "f16" | "bf16"
BANDED = True
MASK_MODE = "lsb"  # "lsb" | "uint8"

_CACHE = {}


def _band_matrix(n, normalize_cols):
    idx = np.arange(n)
    band = (np.abs(idx[:, None] - idx[None, :]) <= PAD).astype(np.float64)
    if normalize_cols:
        cnt = np.minimum(idx + PAD, n - 1) - np.maximum(idx - PAD, 0) + 1
        band = band / cnt[None, :]
    return band


def _build_program(planes=PLANES, reps=1, mm_dtype=None, banded=None,
                   mask_mode=None, ablate=()):
    import concourse.tile as tile
    from concourse import bacc, mybir
    from contextlib import nullcontext

    if mm_dtype is None:
        mm_dtype = MM_DTYPE
    if banded is None:
        banded = BANDED
    if mask_mode is None:
        mask_mode = MASK_MODE
    f32 = mybir.dt.float32
    dt16 = mybir.dt.float16 if mm_dtype == "f16" else mybir.dt.bfloat16
    u16 = mybir.dt.uint16

    nc = bacc.Bacc("TRN2", target_bir_lowering=False, debug=False, num_devices=N_CORES)
    # h-major layout: x[p, h, c, w] (partition dim second) so each plane is
    # one [128 x 4KB] fully-contiguous DMA. Host does the transpose.
    x_d = nc.declare_dram_parameter("x", [planes, 128, FREE], dt16, isOutput=False)
    if mask_mode == "uint8":
        m_d = nc.declare_dram_parameter(
            "mask", [planes, 128, FREE], mybir.dt.uint8, isOutput=False
        )
    bh_d = nc.declare_dram_parameter("bh", [H, H], dt16, isOutput=False)
    bw_d = nc.declare_dram_parameter("bw", [W, W], dt16, isOutput=False)
    out_d = nc.declare_dram_parameter("out", [planes, 128, FREE], dt16, isOutput=True)

    with tile.TileContext(nc) as tc:
        with (
            tc.tile_pool(name="consts", bufs=1) as cpool,
            tc.tile_pool(name="xt", bufs=6) as xpool,
            tc.tile_pool(name="mt", bufs=6) as mpool,
            tc.tile_pool(name="s1b", bufs=4) as s1pool,
            tc.tile_pool(name="ps1", bufs=3, space="PSUM") as ps1pool,
            tc.tile_pool(name="ps2", bufs=5, space="PSUM") as ps2pool,
        ):
            # B constants: stored [128, (chunk, 512)] — partition = row within
            # chunk, free slice c selects row-chunk c.
            bh_t = cpool.tile([128, NCHUNK * H], dt16, tag="bh")
            nc.sync.dma_start(
                out=bh_t[:].rearrange("h (c n) -> h c n", c=NCHUNK),
                in_=bh_d[:].rearrange("(c h) n -> h c n", c=NCHUNK),
            )
            bw_t = cpool.tile([128, NCHUNK * W], dt16, tag="bw")
            nc.sync.dma_start(
                out=bw_t[:].rearrange("h (c n) -> h c n", c=NCHUNK),
                in_=bw_d[:].rearrange("(c h) n -> h c n", c=NCHUNK),
            )

            def mms(ps, lhsT_of, rhs_tile, rhs_base):
                if not banded:
                    for kc in range(NCHUNK):
                        nc.tensor.matmul(
                            ps[:],
                            lhsT=lhsT_of(kc),
                            rhs=rhs_tile[:, rhs_base(kc) : rhs_base(kc) + 512],
                            start=(kc == 0),
                            stop=(kc == NCHUNK - 1),
                        )
                    return
                # Banded: chunk kc only touches output cols [128k-7, 128k+135).
                for kc in range(NCHUNK):
                    lo, hi = 128 * kc, 128 * (kc + 1)
                    segs = []
                    if kc > 0:
                        segs.append((lo - PAD, lo + PAD, False, True))
                    e0 = lo if kc == 0 else lo + PAD
                    e1 = hi if kc == NCHUNK - 1 else hi - PAD
                    segs.append((e0, e1, True, True))
                    if kc < NCHUNK - 1:
                        segs.append((hi - PAD, hi + PAD, True, False))
                    lhsT = lhsT_of(kc)
                    for c0, c1, st, sp in segs:
                        nc.tensor.matmul(
                            ps[:, c0:c1],
                            lhsT=lhsT,
                            rhs=rhs_tile[:, rhs_base(kc) + c0 : rhs_base(kc) + c1],
                            start=st,
                            stop=sp,
                        )

            loop_ctx = (
                tc.For_i(
                    0,
                    reps,
                    1,
                    hint_engines=tuple(
                        getattr(mybir.EngineType, e)
                        for e in ("PE", "Activation", "DVE", "SP", "Pool")
                    ),
                )
                if reps > 1
                else nullcontext()
            )
            with loop_ctx:
              for p in range(planes):
                xt = xpool.tile([128, FREE], dt16, tag="xt")
                if "no_in_dma" not in ablate:
                    nc.sync.dma_start(out=xt[:], in_=x_d[p])
                if mask_mode == "lsb":
                    # predicate = bit0 of the fp16 payload (1 = missing)
                    mt = mpool.tile([128, FREE], u16, tag="mt")
                    if "no_pe" not in ablate:
                        nc.vector.tensor_scalar(
                            out=mt[:],
                            in0=xt[:].bitcast(u16),
                            scalar1=1,
                            scalar2=None,
                            op0=mybir.AluOpType.bitwise_and,
                        )
                else:
                    mt = mpool.tile([128, FREE], mybir.dt.uint8, tag="mt")
                    if "no_in_dma" not in ablate:
                        nc.sync.dma_start(out=mt[:], in_=m_d[p])

                # pass 1: S1T[wc] [128 w, 512 h_out] over h chunks
                s1b = s1pool.tile([128, NCHUNK * H], dt16, tag="s1b")
                for wc in ([] if "no_pe" in ablate else range(NCHUNK)):
                    ps1 = ps1pool.tile([128, H], f32, tag="ps1")
                    mms(
                        ps1,
                        lambda kc: xt[:, kc * W + wc * 128 : kc * W + wc * 128 + 128],
                        bh_t,
                        lambda kc: kc * H,
                    )
                    nc.scalar.copy(s1b[:, wc * H : (wc + 1) * H], ps1[:])

                # pass 2 + in-place blend: overwrite x with the mean where
                # the pixel is missing; xt becomes the output plane.
                for mc in ([] if "no_pe" in ablate else range(NCHUNK)):
                    ps2 = ps2pool.tile([128, W], f32, tag="ps2")
                    mms(
                        ps2,
                        lambda kc: s1b[:, kc * H + mc * 128 : kc * H + mc * 128 + 128],
                        bw_t,
                        lambda kc: kc * W,
                    )
                    nc.vector.copy_predicated(
                        xt[:, mc * W : (mc + 1) * W],
                        mt[:, mc * W : (mc + 1) * W],
                        ps2[:],
                    )
                op_idx = 0 if "out_same" in ablate else p
                nc.sync.dma_start(out=out_d[op_idx], in_=xt[:])
    nc.finalize()
    return nc


def _host_weights(mm_dtype=None):
    if mm_dtype is None:
        mm_dtype = MM_DTYPE
    wt = np.float16 if mm_dtype == "f16" else ml_dtypes.bfloat16
    return (
        _band_matrix(H, True).astype(wt),
        _band_matrix(W, True).astype(wt),
    )


def _get_program():
    if "nc" not in _CACHE:
        _CACHE["nc"] = _build_program()
        _CACHE["bh"], _CACHE["bw"] = _host_weights()
    return _CACHE["nc"], _CACHE["bh"], _CACHE["bw"]


def _to_hmajor(a):
    # [P, (c h), w] -> [P, h, (c w)]
    P = a.shape[0]
    return np.ascontiguousarray(
        a.reshape(P, NCHUNK, 128, W).transpose(0, 2, 1, 3).reshape(P, 128, FREE)
    )


def _from_hmajor(a):
    # [P, h, (c w)] -> [P, (c h), w]
    P = a.shape[0]
    return a.reshape(P, 128, NCHUNK, W).transpose(0, 2, 1, 3).reshape(P, H, W)


def _make_in_maps(x, mask, mm_dtype=None, mask_mode=None):
    """Shard + convert FULL inputs into per-core device input dicts."""
    if mm_dtype is None:
        mm_dtype = MM_DTYPE
    if mask_mode is None:
        mask_mode = MASK_MODE
    wt = np.float16 if mm_dtype == "f16" else ml_dtypes.bfloat16
    P = x.shape[0] * x.shape[1]
    x16 = np.ascontiguousarray(x).astype(wt).reshape(P, H, W)
    missing = (np.ascontiguousarray(mask).reshape(P, H, W) == 0)
    if mask_mode == "lsb":
        xb = x16.view(np.uint16)
        xb &= np.uint16(0xFFFE)
        xb |= missing.astype(np.uint16)
    bh, bw = _host_weights(mm_dtype)
    xs = _to_hmajor(x16).reshape(N_CORES, PLANES, 128, FREE)
    maps = [
        {"x": xs[i], "bh": bh, "bw": bw} for i in range(N_CORES)
    ]
    if mask_mode == "uint8":
        ms = _to_hmajor(missing.astype(np.uint8)).reshape(N_CORES, PLANES, 128, FREE)
        for i in range(N_CORES):
            maps[i]["mask"] = ms[i]
    return maps


def kernel(x: np.ndarray, mask: np.ndarray) -> np.ndarray:
    from concourse.bass_utils import run_bass_kernel_spmd

    nc, _, _ = _get_program()
    in_maps = _make_in_maps(x, mask)
    res = run_bass_kernel_spmd(nc, in_maps, core_ids=list(range(N_CORES)))
    out = np.stack([
        _from_hmajor(res.results[i]["out"]).astype(np.float32)
        for i in range(N_CORES)
    ])
    return out.reshape(-1, CHANNELS, H, W)


# revision 33
# speedup vs baseline: 2.2048x; 1.1765x over previous
"""LocalMeanInpainter Trainium2 kernel.

out = x*mask + (box15(x)/box15(ones))*(1-mask)  over (32,3,512,512) f32.

Strategy: data-parallel over batch (4 images x 3 channels = 12 planes of
512x512 per core, 8 cores). Per plane, the 15x15 box mean is separable:
mean = BHn^T @ X @ BWn with BHn/BWn the 0/1 banded matrices (|i-j|<=7)
column-normalized by the 1-D in-bounds counts (cnt = outer(ch, cw) exactly).
Both passes run on the PE tensor engine:
  pass1: S1T[w, h_out] = sum_h X[h, w] * BH[h, h_out]   (X chunk stationary)
  pass2: S2[h_out, w_out] = sum_w S1T[w, h_out] * BW[w, w_out]

Perf notes (HW-measured path: 104us f32r-dense baseline -> 63us fp16-banded
-> this version). The kernel is DMA-bound, so everything targets HBM bytes:
 - x / out / B-weights travel as fp16 (tolerance is 2e-2; fp16 adds ~2e-4).
 - mask travels as the LSB of x's fp16 mantissa (host-encoded): missing
   pixels have bit0 set, kept pixels bit0 cleared (<=1ulp noise). A single
   DVE tensor_scalar(bitwise_and) per plane recovers the predicate, so the
   3.1MB/core uint8 mask DMA disappears entirely.
 - host ships planes in h-major layout [plane, h(128), chunk, w] so every
   plane is ONE [128 x 4KB-contiguous] DMA in and out.
 - banded matmuls: only the |i-j|<=7 band of B contributes, so each
   contraction chunk only touches ~139 of 512 output columns (fp16 runs
   1 cycle/row at any width; f32r needed >=256-wide to hit full rate).
 - blend is done IN PLACE into the x tile: copy_predicated overwrites x
   with the PSUM mean where the mask bit says "missing", then the x tile
   is DMA'd out directly. No staging copy of the mean.
"""

import numpy as np
import ml_dtypes

H = 512
W = 512
WINDOW = 15
PAD = 7
N_CORES = 8
IMGS_PER_CORE = 4
CHANNELS = 3
PLANES = IMGS_PER_CORE * CHANNELS  # 12
NCHUNK = H // 128  # 4
FREE = NCHUNK * W  # 2048 free elems per partition per plane

MM_DTYPE = "f16"  # "f16" | "bf16"
BANDED = "merged"  # "merged" | True | False
MASK_MODE = "lsb"  # "lsb" | "uint8"
OUT_DMA = "mixed"  # "sp"|"act"|"pool"|"alt"|"mixed": queue for store DMAs
# ("mixed" = last 2 planes' stores on the Act HWDGE queue so the SP stream's
#  loop-back edge never stalls on the final blend; the rest on SP)
S1_ENGINE = "act"  # "act" | "pool": engine for the pass-1 PSUM->SBUF copies
PREFETCH = 12  # x loads dispatched this many planes ahead of the store
STORE_SPLIT = False  # True: store each 128-row chunk right after its blend
LOAD_GROUP = 1  # planes loaded per DMA (1 or 3): fewer, larger dispatches

_CACHE = {}


def _band_matrix(n, normalize_cols):
    idx = np.arange(n)
    band = (np.abs(idx[:, None] - idx[None, :]) <= PAD).astype(np.float64)
    if normalize_cols:
        cnt = np.minimum(idx + PAD, n - 1) - np.maximum(idx - PAD, 0) + 1
        band = band / cnt[None, :]
    return band


def _build_program(planes=PLANES, reps=1, mm_dtype=None, banded=None,
                   mask_mode=None, out_dma=None, s1_engine=None,
                   prefetch=None, load_group=None, ablate=()):
    import concourse.tile as tile
    from concourse import bacc, mybir
    from contextlib import nullcontext

    if mm_dtype is None:
        mm_dtype = MM_DTYPE
    if banded is None:
        banded = BANDED
    if mask_mode is None:
        mask_mode = MASK_MODE
    if out_dma is None:
        out_dma = OUT_DMA
    if s1_engine is None:
        s1_engine = S1_ENGINE
    if prefetch is None:
        prefetch = PREFETCH
    store_split = STORE_SPLIT if "store_split" not in ablate else True
    if load_group is None:
        load_group = LOAD_GROUP
    G = load_group if planes % load_group == 0 else 1
    f32 = mybir.dt.float32
    dt16 = mybir.dt.float16 if mm_dtype == "f16" else mybir.dt.bfloat16
    u16 = mybir.dt.uint16

    nc = bacc.Bacc("TRN2", target_bir_lowering=False, debug=False, num_devices=N_CORES)
    # h-major layout: x[g, h, (plane-in-group, c, w)] (partition dim second)
    # so each load group is one [128 x G*4KB-contiguous-per-partition] DMA.
    # Host does the transpose.
    x_d = nc.declare_dram_parameter(
        "x", [planes // G, 128, G * FREE], dt16, isOutput=False
    )
    if mask_mode == "uint8":
        m_d = nc.declare_dram_parameter(
            "mask", [planes // G, 128, G * FREE], mybir.dt.uint8, isOutput=False
        )
    bh_d = nc.declare_dram_parameter("bh", [H, H], dt16, isOutput=False)
    bw_d = nc.declare_dram_parameter("bw", [W, W], dt16, isOutput=False)
    out_d = nc.declare_dram_parameter("out", [planes, 128, FREE], dt16, isOutput=True)

    ngroups = planes // G
    nbufs = ngroups if prefetch >= ngroups else 6
    with tile.TileContext(nc) as tc:
        with (
            tc.tile_pool(name="consts", bufs=1) as cpool,
            tc.tile_pool(name="xt", bufs=nbufs) as xpool,
            tc.tile_pool(name="mt", bufs=nbufs) as mpool,
            tc.tile_pool(name="s1b", bufs=4) as s1pool,
            tc.tile_pool(name="ps1", bufs=3, space="PSUM") as ps1pool,
            tc.tile_pool(name="ps2", bufs=5, space="PSUM") as ps2pool,
        ):
            # B constants: stored [128, (chunk, 512)] — partition = row within
            # chunk, free slice c selects row-chunk c.
            bh_t = cpool.tile([128, NCHUNK * H], dt16, tag="bh")
            nc.sync.dma_start(
                out=bh_t[:].rearrange("h (c n) -> h c n", c=NCHUNK),
                in_=bh_d[:].rearrange("(c h) n -> h c n", c=NCHUNK),
            )
            bw_t = cpool.tile([128, NCHUNK * W], dt16, tag="bw")
            nc.sync.dma_start(
                out=bw_t[:].rearrange("h (c n) -> h c n", c=NCHUNK),
                in_=bw_d[:].rearrange("(c h) n -> h c n", c=NCHUNK),
            )

            def mms(ps, lhsT_of, rhs_tile, rhs_base):
                if not banded:
                    for kc in range(NCHUNK):
                        nc.tensor.matmul(
                            ps[:],
                            lhsT=lhsT_of(kc),
                            rhs=rhs_tile[:, rhs_base(kc) : rhs_base(kc) + 512],
                            start=(kc == 0),
                            stop=(kc == NCHUNK - 1),
                        )
                    return
                # Banded: chunk kc only touches output cols [128k-7, 128k+135).
                for kc in range(NCHUNK):
                    lo, hi = 128 * kc, 128 * (kc + 1)
                    segs = []
                    if banded == "merged":
                        # 2 matmuls per chunk (vs 3): the overlap region
                        # [lo-7, lo+7) accumulates onto the previous chunk's
                        # tail, the rest is a fresh start. Each ldweights is
                        # paid per matmul (walrus runs with ldw-opt off), so
                        # fewer matmuls = less PE time. start/stop no longer
                        # form clean per-column groups -> skip_group_check.
                        if kc > 0:
                            segs.append((lo - PAD, lo + PAD, False, True))
                        e1 = hi if kc == NCHUNK - 1 else hi + PAD
                        segs.append((lo + PAD if kc > 0 else lo, e1, True,
                                     kc == NCHUNK - 1))
                    else:
                        if kc > 0:
                            segs.append((lo - PAD, lo + PAD, False, True))
                        e0 = lo if kc == 0 else lo + PAD
                        e1 = hi if kc == NCHUNK - 1 else hi - PAD
                        segs.append((e0, e1, True, True))
                        if kc < NCHUNK - 1:
                            segs.append((hi - PAD, hi + PAD, True, False))
                    lhsT = lhsT_of(kc)
                    for c0, c1, st, sp in segs:
                        nc.tensor.matmul(
                            ps[:, c0:c1],
                            lhsT=lhsT,
                            rhs=rhs_tile[:, rhs_base(kc) + c0 : rhs_base(kc) + c1],
                            start=st,
                            stop=sp,
                            skip_group_check=banded == "merged",
                        )

            loop_ctx = (
                tc.For_i(
                    0,
                    reps,
                    1,
                    hint_engines=tuple(
                        getattr(mybir.EngineType, e)
                        for e in ("PE", "Activation", "DVE", "SP", "Pool")
                    ),
                )
                if reps > 1
                else nullcontext()
            )
            xts = {}

            def load_group(gi):
                # x load + (lsb) predicate extraction. Dispatched ahead of
                # the previous planes' store DMAs so the SP stream never
                # stalls the loads behind a store that waits on the blend.
                xt = xpool.tile([128, G * FREE], dt16, tag="xt")
                if "no_in_dma" not in ablate:
                    nc.sync.dma_start(out=xt[:], in_=x_d[gi])
                if mask_mode == "lsb":
                    mt = mpool.tile([128, G * FREE], u16, tag="mt")
                    if "no_pe" not in ablate and "no_and" not in ablate:
                        nc.vector.tensor_scalar(
                            out=mt[:],
                            in0=xt[:].bitcast(u16),
                            scalar1=1,
                            scalar2=None,
                            op0=mybir.AluOpType.bitwise_and,
                        )
                else:
                    mt = mpool.tile([128, G * FREE], mybir.dt.uint8, tag="mt")
                    if "no_in_dma" not in ablate:
                        nc.sync.dma_start(out=mt[:], in_=m_d[gi])
                xts[gi] = (xt, mt)

            s1_copy = nc.gpsimd.tensor_copy if s1_engine == "pool" else nc.scalar.copy
            pf_groups = (prefetch + G - 1) // G if prefetch < planes else ngroups
            with loop_ctx:
              for p in range(-pf_groups * G, planes):
                if p % G == 0 and (p + pf_groups * G) // G < ngroups:
                    load_group((p + pf_groups * G) // G)
                if p < 0:
                    continue
                xtg_full, mtg_full = xts[p // G]
                j = p % G
                xt = xtg_full[:, j * FREE : (j + 1) * FREE]
                mt = mtg_full[:, j * FREE : (j + 1) * FREE]
                if j == G - 1:
                    xts.pop(p // G, None)

                # pass 1: S1T[wc] [128 w, 512 h_out] over h chunks
                s1b = s1pool.tile([128, NCHUNK * H], dt16, tag="s1b")
                for wc in ([] if "no_pe" in ablate else range(NCHUNK)):
                    ps1 = ps1pool.tile([128, H], f32, tag="ps1")
                    mms(
                        ps1,
                        lambda kc: xt[:, kc * W + wc * 128 : kc * W + wc * 128 + 128],
                        bh_t,
                        lambda kc: kc * H,
                    )
                    s1_copy(s1b[:, wc * H : (wc + 1) * H], ps1[:])

                # pass 2 + in-place blend: overwrite x with the mean where
                # the pixel is missing; xt becomes the output plane.
                op_idx = 0 if "out_same" in ablate else p
                use_act = (
                    out_dma == "act"
                    or (out_dma == "alt" and p % 2 == 0)
                    or (out_dma == "mixed" and p >= planes - 2)
                    or (out_dma == "mixed4" and p >= planes - 4)
                )
                out_eng = (
                    nc.gpsimd if out_dma == "pool"
                    else nc.scalar if use_act
                    else nc.sync
                )
                for mc in ([] if "no_pe" in ablate else range(NCHUNK)):
                    ps2 = ps2pool.tile([128, W], f32, tag="ps2")
                    mms(
                        ps2,
                        lambda kc: s1b[:, kc * H + mc * 128 : kc * H + mc * 128 + 128],
                        bw_t,
                        lambda kc: kc * W,
                    )
                    if "no_blend" not in ablate:
                        nc.vector.copy_predicated(
                            xt[:, mc * W : (mc + 1) * W],
                            mt[:, mc * W : (mc + 1) * W],
                            ps2[:],
                        )
                    if store_split:
                        out_eng.dma_start(
                            out=out_d[op_idx, :, mc * W : (mc + 1) * W],
                            in_=xt[:, mc * W : (mc + 1) * W],
                        )
                if not store_split:
                    out_eng.dma_start(out=out_d[op_idx], in_=xt[:])
    nc.finalize()
    return nc


def _host_weights(mm_dtype=None):
    if mm_dtype is None:
        mm_dtype = MM_DTYPE
    wt = np.float16 if mm_dtype == "f16" else ml_dtypes.bfloat16
    return (
        _band_matrix(H, True).astype(wt),
        _band_matrix(W, True).astype(wt),
    )


def _get_program():
    if "nc" not in _CACHE:
        _CACHE["nc"] = _build_program()
        _CACHE["bh"], _CACHE["bw"] = _host_weights()
    return _CACHE["nc"], _CACHE["bh"], _CACHE["bw"]


def _to_hmajor(a):
    # [P, (c h), w] -> [P, h, (c w)]
    P = a.shape[0]
    return np.ascontiguousarray(
        a.reshape(P, NCHUNK, 128, W).transpose(0, 2, 1, 3).reshape(P, 128, FREE)
    )


def _from_hmajor(a):
    # [P, h, (c w)] -> [P, (c h), w]
    P = a.shape[0]
    return a.reshape(P, 128, NCHUNK, W).transpose(0, 2, 1, 3).reshape(P, H, W)


def _group_planes(a, g):
    # [NC, P, 128, F] -> [NC, P//g, 128, g*F]
    NC, P, _, F = a.shape
    return np.ascontiguousarray(
        a.reshape(NC, P // g, g, 128, F).transpose(0, 1, 3, 2, 4)
    ).reshape(NC, P // g, 128, g * F)


def _make_in_maps(x, mask, mm_dtype=None, mask_mode=None, load_group=None):
    """Shard + convert FULL inputs into per-core device input dicts."""
    if mm_dtype is None:
        mm_dtype = MM_DTYPE
    if mask_mode is None:
        mask_mode = MASK_MODE
    if load_group is None:
        load_group = LOAD_GROUP
    wt = np.float16 if mm_dtype == "f16" else ml_dtypes.bfloat16
    P = x.shape[0] * x.shape[1]
    x16 = np.ascontiguousarray(x).astype(wt).reshape(P, H, W)
    missing = (np.ascontiguousarray(mask).reshape(P, H, W) == 0)
    if mask_mode == "lsb":
        xb = x16.view(np.uint16)
        xb &= np.uint16(0xFFFE)
        xb |= missing.astype(np.uint16)
    bh, bw = _host_weights(mm_dtype)
    xs = _to_hmajor(x16).reshape(N_CORES, PLANES, 128, FREE)
    if load_group > 1:
        xs = _group_planes(xs, load_group)
    maps = [
        {"x": xs[i], "bh": bh, "bw": bw} for i in range(N_CORES)
    ]
    if mask_mode == "uint8":
        ms = _to_hmajor(missing.astype(np.uint8)).reshape(N_CORES, PLANES, 128, FREE)
        if load_group > 1:
            ms = _group_planes(ms, load_group)
        for i in range(N_CORES):
            maps[i]["mask"] = ms[i]
    return maps


def kernel(x: np.ndarray, mask: np.ndarray) -> np.ndarray:
    from concourse.bass_utils import run_bass_kernel_spmd

    nc, _, _ = _get_program()
    in_maps = _make_in_maps(x, mask)
    res = run_bass_kernel_spmd(nc, in_maps, core_ids=list(range(N_CORES)))
    out = np.stack([
        _from_hmajor(res.results[i]["out"]).astype(np.float32)
        for i in range(N_CORES)
    ])
    return out.reshape(-1, CHANNELS, H, W)
